# revision 4
# baseline (speedup 1.0000x reference)
"""Trainium2 Bass kernel for nn_AttentionModel (greedy pointer-attention decode).

Contract: kernel(**inputs) takes FULL inputs (B=1024), shards batch across 8
NeuronCores (128 items each, SPMD), runs the 199-step greedy decode on-device,
returns full (1024, 199, 200) float32 log_p.

Per-core dataflow (batch-on-partitions, b=128):
  precompute: emb2 = emb + pref -> DRAM;  kvl = emb2 @ W_node -> DRAM (gK|gV|lK)
              fixed2 = mean(emb2) @ W_fixed + first @ W_step[:256]
  per step  : stream kvl chunks from HBM; DVE does batched dot-products
              (multiply + strided reduce); ACT does exp/tanh/ln; PE does the
              shared-weight matmuls (cur @ W_step[256:], glimpse @ W_out) and
              transposes; argmax via DVE max/max_index; visited mask kept as a
              0/-1e9 addend; cur_emb gathered by indirect DMA with on-device
              computed row offsets.
"""
import numpy as np

import concourse.bass as bass
from concourse import bacc
import concourse.tile as tile
from concourse import mybir
from concourse.bass import IndirectOffsetOnAxis
from concourse.bass_utils import run_bass_kernel_spmd

dt = mybir.dt
F32 = dt.float32
AX = mybir.AxisListType
OP = mybir.AluOpType
ACTF = mybir.ActivationFunctionType

B, N, D, H = 1024, 200, 256, 8
d = D // H                      # 32
NCORES = 8
BS = B // NCORES                # 128 items per core
T = N - 1                       # 199 decode steps
START = 24
NEG = -1e9
CHUNK = 20                      # n-chunk for streaming kvl
NCH = N // CHUNK                # 10 chunks
ISD = 1.0 / np.sqrt(d).item()   # 1/sqrt(32)
ISD32 = float(np.float32(1.0 / np.sqrt(32.0)))
ISD256 = 0.0625                 # 1/sqrt(256), exact


def _build():
    nc = bacc.Bacc("TRN2", target_bir_lowering=False, debug=False)

    emb_in = nc.dram_tensor("embeddings", [BS, N, D], F32, kind="ExternalInput").ap()
    pref_in = nc.dram_tensor("pref_embed", [D], F32, kind="ExternalInput").ap()
    wnode_in = nc.dram_tensor("W_node", [D, 3 * D], F32, kind="ExternalInput").ap()
    wfix_in = nc.dram_tensor("W_fixed", [D, D], F32, kind="ExternalInput").ap()
    wstep_in = nc.dram_tensor("W_step", [2 * D, D], F32, kind="ExternalInput").ap()
    wout_in = nc.dram_tensor("W_out", [D, D], F32, kind="ExternalInput").ap()

    out = nc.dram_tensor("log_p", [BS, T * N], F32, kind="ExternalOutput").ap()

    emb2_d = nc.dram_tensor("emb2_d", [BS * N, D], F32).ap()
    kvl_d = nc.dram_tensor("kvl_d", [BS, N, 3 * D], F32).ap()

    with tile.TileContext(nc) as tc:
        with (
            tc.tile_pool(name="wpool", bufs=1) as wpool,      # persistent weights/state
            tc.tile_pool(name="stream", bufs=4) as stream,    # kvl chunks
            tc.tile_pool(name="prod", bufs=2) as prodp,       # TT products
            tc.tile_pool(name="work", bufs=2) as work,        # small transient tiles
            tc.tile_pool(name="psum", bufs=2, space="PSUM") as psp,
            tc.tile_pool(name="psum1", bufs=2, space="PSUM") as psp1,
        ):
            # ---------------- persistent tiles ----------------
            wn_sb = wpool.tile([128, 2, 3 * D], F32)    # W_node as [c-tile, 2, 768]
            nc.sync.dma_start(wn_sb[:, 0, :], wnode_in[0:128, :])
            nc.sync.dma_start(wn_sb[:, 1, :], wnode_in[128:256, :])
            w2_sb = wpool.tile([128, 2, D], F32)        # W_step[256:512] c-tiles
            nc.sync.dma_start(w2_sb[:, 0, :], wstep_in[256:384, :])
            nc.sync.dma_start(w2_sb[:, 1, :], wstep_in[384:512, :])
            wo_sb = wpool.tile([128, 2, D], F32)        # W_out c-tiles
            nc.sync.dma_start(wo_sb[:, 0, :], wout_in[0:128, :])
            nc.sync.dma_start(wo_sb[:, 1, :], wout_in[128:256, :])
            wf_sb = wpool.tile([128, 2, D], F32)        # W_fixed c-tiles
            nc.sync.dma_start(wf_sb[:, 0, :], wfix_in[0:128, :])
            nc.sync.dma_start(wf_sb[:, 1, :], wfix_in[128:256, :])
            ws1_sb = wpool.tile([128, 2, D], F32)       # W_step[0:256] c-tiles
            nc.sync.dma_start(ws1_sb[:, 0, :], wstep_in[0:128, :])
            nc.sync.dma_start(ws1_sb[:, 1, :], wstep_in[128:256, :])

            pref_sb = wpool.tile([128, D], F32)
            nc.sync.dma_start(
                pref_sb[:],
                pref_in.rearrange("(o f) -> o f", o=1).broadcast_to([128, D]),
            )

            ident = wpool.tile([128, 128], F32)         # identity for PE transpose
            io_c = wpool.tile([128, 128], dt.int32)
            nc.gpsimd.iota(io_c[:], pattern=[[1, 128]], channel_multiplier=0)
            io_r = wpool.tile([128, 1], dt.int32)
            nc.gpsimd.iota(io_r[:], pattern=[[0, 1]], channel_multiplier=1)
            id_i = wpool.tile([128, 128], dt.int32)
            nc.vector.tensor_tensor(id_i[:], io_c[:], io_r[:].broadcast_to([128, 128]), op=OP.is_equal)
            nc.vector.tensor_copy(ident[:], id_i[:])

            iota_n = wpool.tile([128, N], dt.int32)     # 0..199 per partition
            nc.gpsimd.iota(iota_n[:], pattern=[[1, N]], channel_multiplier=0)
            iota_row = wpool.tile([128, 1], dt.int32)   # p*N
            nc.gpsimd.iota(iota_row[:], pattern=[[0, 1]], channel_multiplier=N)

            amask = wpool.tile([128, N], F32)           # visited addend 0/-1e9
            nc.vector.memset(amask[:], 0.0)
            nc.vector.memset(amask[:, START:START + 1], NEG)

            fixed2 = wpool.tile([128, D], F32)
            first_sb = wpool.tile([128, D], F32)
            q_sb = wpool.tile([128, D], F32)
            cur_sb = wpool.tile([128, D], F32)

            # ---------------- precompute: emb2 + kvl ----------------
            emb_rows = emb_in.rearrange("b n c -> (b n) c")   # [25600, 256]
            ROWT = BS * N // 128                              # 200 row-tiles

            def pre_body(rt):
                erow = work.tile([128, D], F32, tag="erow")
                nc.sync.dma_start(erow[:], emb_rows[bass.ds(rt * 128, 128), :])
                e2 = work.tile([128, D], F32, tag="e2")
                nc.vector.tensor_tensor(e2[:], erow[:], pref_sb[:], op=OP.add)
                nc.sync.dma_start(emb2_d[bass.ds(rt * 128, 128), :], e2[:])
                # transpose e2 -> e2T (2 c-tiles)
                e2T = work.tile([128, 2, 128], F32, tag="e2T")
                for ci in range(2):
                    tp = psp1.tile([128, 128], F32, tag="tp")
                    nc.tensor.transpose(tp[:], e2[:, ci * 128:(ci + 1) * 128], ident[:])
                    nc.vector.tensor_copy(e2T[:, ci, :], tp[:])
                # kvl row-tile = e2 @ W_node  (f split 2x384)
                kv = work.tile([128, 3 * D], F32, tag="kv")
                for fh in range(2):
                    pm = psp.tile([128, 384], F32, tag="ps")
                    nc.tensor.matmul(pm[:], e2T[:, 0, :], wn_sb[:, 0, fh * 384:(fh + 1) * 384], start=True, stop=False)
                    nc.tensor.matmul(pm[:], e2T[:, 1, :], wn_sb[:, 1, fh * 384:(fh + 1) * 384], start=False, stop=True)
                    nc.vector.tensor_copy(kv[:, fh * 384:(fh + 1) * 384], pm[:])
                nc.sync.dma_start(kvl_d.rearrange("b n c -> (b n) c")[bass.ds(rt * 128, 128), :], kv[:])

            tc.For_i_unrolled(0, ROWT, 1, pre_body, max_unroll=4)

            # ---------------- fixed2 ----------------
            macc = wpool.tile([128, D], F32)
            emb2_bnc = emb2_d.rearrange("(b n) c -> b n c", b=BS)
            for c in range(NCH):
                ech = stream.tile([128, CHUNK, D], F32, tag="stream")
                nc.sync.dma_start(ech[:], emb2_bnc[:, c * CHUNK:(c + 1) * CHUNK, :])
                part = work.tile([128, D], F32, tag="mpart")
                nc.vector.tensor_reduce(part[:], ech[:].transpose([0, 2, 1]), axis=AX.X, op=OP.add)
                if c == 0:
                    nc.vector.tensor_copy(macc[:], part[:])
                else:
                    nc.vector.tensor_tensor(macc[:], macc[:], part[:], op=OP.add)
            nc.vector.tensor_scalar(macc[:], macc[:], 1.0 / N, None, op0=OP.mult)
            nc.sync.dma_start(first_sb[:], emb2_bnc[:, START, :])

            fT = work.tile([128, 2, 128], F32, tag="fT")
            mT = work.tile([128, 2, 128], F32, tag="mT")
            for ci in range(2):
                tp = psp1.tile([128, 128], F32, tag="tp")
                nc.tensor.transpose(tp[:], macc[:, ci * 128:(ci + 1) * 128], ident[:])
                nc.vector.tensor_copy(mT[:, ci, :], tp[:])
                tp2 = psp1.tile([128, 128], F32, tag="tp")
                nc.tensor.transpose(tp2[:], first_sb[:, ci * 128:(ci + 1) * 128], ident[:])
                nc.vector.tensor_copy(fT[:, ci, :], tp2[:])
            pf = psp.tile([128, D], F32, tag="ps")
            nc.tensor.matmul(pf[:], mT[:, 0, :], wf_sb[:, 0, :], start=True, stop=False)
            nc.tensor.matmul(pf[:], mT[:, 1, :], wf_sb[:, 1, :], start=False, stop=False)
            nc.tensor.matmul(pf[:], fT[:, 0, :], ws1_sb[:, 0, :], start=False, stop=False)
            nc.tensor.matmul(pf[:], fT[:, 1, :], ws1_sb[:, 1, :], start=False, stop=True)
            nc.vector.tensor_copy(fixed2[:], pf[:])

            # q(t=0): cur = first_emb
            nc.vector.tensor_copy(cur_sb[:], first_sb[:])

            def q_from_cur():
                cT = work.tile([128, 2, 128], F32, tag="cT")
                for ci in range(2):
                    tp = psp1.tile([128, 128], F32, tag="tp")
                    nc.tensor.transpose(tp[:], cur_sb[:, ci * 128:(ci + 1) * 128], ident[:])
                    nc.scalar.copy(cT[:, ci, :], tp[:])
                pq = psp.tile([128, D], F32, tag="ps")
                nc.tensor.matmul(pq[:], cT[:, 0, :], w2_sb[:, 0, :], start=True, stop=False)
                nc.tensor.matmul(pq[:], cT[:, 1, :], w2_sb[:, 1, :], start=False, stop=True)
                nc.scalar.activation(q_sb[:], pq[:], ACTF.Copy)
                nc.vector.tensor_tensor(q_sb[:], q_sb[:], fixed2[:], op=OP.add)

            q_from_cur()

            # ---------------- decode steps ----------------
            compat = wpool.tile([128, H, N], F32)
            attn = wpool.tile([128, H, N], F32)
            logits = wpool.tile([128, N], F32)
            gl_part = wpool.tile([128, NCH, D], F32)
            glimpse = wpool.tile([128, D], F32)

            def step_body(s):
                qb = q_sb[:].rearrange("p (o f) -> p o f", o=1).broadcast_to([128, CHUNK, D])
                # --- compat: per-head dots with gK ---
                for c in range(NCH):
                    kc = stream.tile([128, CHUNK, D], F32, tag="stream")
                    nc.sync.dma_start(kc[:], kvl_d[:, c * CHUNK:(c + 1) * CHUNK, 0:D])
                    pr = prodp.tile([128, CHUNK, D], F32, tag="prod")
                    nc.gpsimd.tensor_tensor(pr[:], kc[:], qb, op=OP.mult)
                    nc.vector.tensor_reduce(
                        compat[:, :, c * CHUNK:(c + 1) * CHUNK].transpose([0, 2, 1]),
                        pr[:].rearrange("p n (h e) -> p n h e", h=H),
                        axis=AX.X, op=OP.add)
                # scale + mask + softmax over n (per head)
                ab = amask[:].rearrange("p (o n) -> p o n", o=1).broadcast_to([128, H, N])
                nc.vector.tensor_scalar(compat[:], compat[:], ISD32, None, op0=OP.mult)
                nc.vector.tensor_tensor(compat[:], compat[:], ab, op=OP.add)
                mh = work.tile([128, H], F32, tag="mh")
                nc.vector.tensor_reduce(mh[:], compat[:], axis=AX.X, op=OP.max)
                nc.vector.tensor_tensor(
                    compat[:], compat[:],
                    mh[:].rearrange("p (h o) -> p h o", o=1).broadcast_to([128, H, N]),
                    op=OP.subtract)
                nc.scalar.activation(attn[:], compat[:], ACTF.Exp)
                sh = work.tile([128, H], F32, tag="sh")
                nc.vector.tensor_reduce(sh[:], attn[:], axis=AX.X, op=OP.add)
                rh = work.tile([128, H], F32, tag="rh")
                nc.vector.reciprocal(rh[:], sh[:])
                nc.vector.tensor_tensor(
                    attn[:], attn[:],
                    rh[:].rearrange("p (h o) -> p h o", o=1).broadcast_to([128, H, N]),
                    op=OP.mult)
                # --- glimpse: attn-weighted gV ---
                for c in range(NCH):
                    vc = stream.tile([128, CHUNK, D], F32, tag="stream")
                    nc.sync.dma_start(vc[:], kvl_d[:, c * CHUNK:(c + 1) * CHUNK, D:2 * D])
                    av = attn[:, :, c * CHUNK:(c + 1) * CHUNK].transpose([0, 2, 1]) \
                        .rearrange("p n (h o) -> p n h o", o=1).broadcast_to([128, CHUNK, H, d])
                    pr = prodp.tile([128, CHUNK, D], F32, tag="prod")
                    nc.gpsimd.tensor_tensor(pr[:].rearrange("p n (h e) -> p n h e", h=H), vc[:].rearrange("p n (h e) -> p n h e", h=H), av, op=OP.mult)
                    nc.vector.tensor_reduce(gl_part[:, c, :], pr[:].transpose([0, 2, 1]), axis=AX.X, op=OP.add)
                nc.vector.tensor_reduce(glimpse[:], gl_part[:].transpose([0, 2, 1]), axis=AX.X, op=OP.add)
                # g = glimpse @ W_out
                gT = work.tile([128, 2, 128], F32, tag="gT")
                for ci in range(2):
                    tp = psp1.tile([128, 128], F32, tag="tp")
                    nc.tensor.transpose(tp[:], glimpse[:, ci * 128:(ci + 1) * 128], ident[:])
                    nc.scalar.copy(gT[:, ci, :], tp[:])
                pg = psp.tile([128, D], F32, tag="ps")
                nc.tensor.matmul(pg[:], gT[:, 0, :], wo_sb[:, 0, :], start=True, stop=False)
                nc.tensor.matmul(pg[:], gT[:, 1, :], wo_sb[:, 1, :], start=False, stop=True)
                g_sb = work.tile([128, D], F32, tag="g_sb")
                nc.scalar.copy(g_sb[:], pg[:])
                gb = g_sb[:].rearrange("p (o f) -> p o f", o=1).broadcast_to([128, CHUNK, D])
                # --- logits: g . lK ---
                for c in range(NCH):
                    lc = stream.tile([128, CHUNK, D], F32, tag="stream")
                    nc.sync.dma_start(lc[:], kvl_d[:, c * CHUNK:(c + 1) * CHUNK, 2 * D:3 * D])
                    pr = prodp.tile([128, CHUNK, D], F32, tag="prod")
                    nc.gpsimd.tensor_tensor(pr[:], lc[:], gb, op=OP.mult)
                    nc.vector.tensor_reduce(logits[:, c * CHUNK:(c + 1) * CHUNK], pr[:], axis=AX.X, op=OP.add)
                # tanh clip, mask, log_softmax
                lgt = work.tile([128, N], F32, tag="lgt")
                nc.scalar.activation(lgt[:], logits[:], ACTF.Tanh, scale=ISD256)
                nc.vector.tensor_scalar(logits[:], lgt[:], 10.0, None, op0=OP.mult)
                nc.vector.tensor_tensor(logits[:], logits[:], amask[:], op=OP.add)
                m1 = work.tile([128, 1], F32, tag="m1")
                nc.vector.tensor_reduce(m1[:], logits[:], axis=AX.X, op=OP.max)
                shl = work.tile([128, N], F32, tag="shl")
                nc.vector.tensor_tensor(shl[:], logits[:], m1[:].broadcast_to([128, N]), op=OP.subtract)
                pexp = work.tile([128, N], F32, tag="pexp")
                s1 = work.tile([128, 1], F32, tag="s1")
                nc.scalar.activation(pexp[:], shl[:], ACTF.Exp, accum_out=s1[:])
                ls = work.tile([128, 1], F32, tag="ls")
                nc.scalar.activation(ls[:], s1[:], ACTF.Ln)
                lp = work.tile([128, N], F32, tag="lp")
                nc.vector.tensor_tensor(lp[:], shl[:], ls[:].broadcast_to([128, N]), op=OP.subtract)
                nc.sync.dma_start(out[:, bass.ds(s * N, N)], lp[:])
                # --- argmax + state update ---
                mx8 = work.tile([128, 8], F32, tag="mx8")
                nc.vector.max(mx8[:], logits[:])
                ix8 = work.tile([128, 8], dt.uint32, tag="ix8")
                nc.vector.max_index(ix8[:], mx8[:], logits[:])
                sel = work.tile([128, 1], dt.int32, tag="sel")
                nc.vector.tensor_copy(sel[:], ix8[:, 0:1])
                ohi = work.tile([128, N], dt.int32, tag="ohi")
                nc.vector.tensor_tensor(ohi[:], iota_n[:], sel[:].broadcast_to([128, N]), op=OP.is_equal)
                ohf = work.tile([128, N], F32, tag="ohf")
                nc.vector.tensor_copy(ohf[:], ohi[:])
                nc.vector.tensor_scalar(ohf[:], ohf[:], NEG, None, op0=OP.mult)
                nc.vector.tensor_tensor(amask[:], amask[:], ohf[:], op=OP.add)
                # gather next cur + q
                offs = work.tile([128, 1], dt.int32, tag="offs")
                nc.vector.tensor_tensor(offs[:], iota_row[:], sel[:], op=OP.add)
                nc.gpsimd.indirect_dma_start(
                    out=cur_sb[:], out_offset=None,
                    in_=emb2_d, in_offset=IndirectOffsetOnAxis(ap=offs[:], axis=0))
                q_from_cur()

            tc.For_i_unrolled(0, T, 1, step_body, max_unroll=4)

    nc.compile()
    return nc


_CACHE = {}


def kernel(**inputs) -> np.ndarray:
    if "nc" not in _CACHE:
        _CACHE["nc"] = _build()
    nc = _CACHE["nc"]

    emb = np.ascontiguousarray(np.asarray(inputs["embeddings"], np.float32))
    shared = {
        "pref_embed": np.asarray(inputs["pref_embed"], np.float32),
        "W_node": np.asarray(inputs["W_node"], np.float32),
        "W_fixed": np.asarray(inputs["W_fixed"], np.float32),
        "W_step": np.asarray(inputs["W_step"], np.float32),
        "W_out": np.asarray(inputs["W_out"], np.float32),
    }
    in_maps = []
    for i in range(NCORES):
        m = {"embeddings": emb[i * BS:(i + 1) * BS]}
        m.update(shared)
        in_maps.append(m)

    res = run_bass_kernel_spmd(nc, in_maps, list(range(NCORES)))
    outs = [res.results[i]["log_p"].reshape(BS, T, N) for i in range(NCORES)]
    return np.concatenate(outs, axis=0)


if __name__ == "__main__":
    z = np.load("inputs.npz")
    inp = {k: z[k] for k in z.files}
    o = kernel(**inp)
    print("kernel output", o.shape, o.dtype)
    np.save("kernel_out.npy", o)
